# revision 1
# baseline (speedup 1.0000x reference)
"""CircleLoss (nn_CircleLoss_17884243820936) — Trainium2 Bass kernel, 8 NeuronCores.

Math (forward value of the reference):
  x̂ = L2-normalized embeddings, sim = x̂ x̂ᵀ, t = 16·sim  (γ=256, √γ=16)
  logit_p = -γ·relu(1+m-sim)·(sim-(1-m)) = (t-16)² - 16   (exact for sim ≤ 1+m)
  logit_n =  γ·relu(sim+m)·(sim-m)      = max(t,-4)² - 16 (clamp encodes relu)
  loss = softplus(lse_pos(logit_p) + lse_neg(logit_n))

Sharding: rows sorted by label so all same-label pairs live within 128
rows of each other. The upper triangle is split into per-row-chunk
"bands" (256 cols at the diagonal: all pos pairs + near-diag neg pairs) and
pure-neg "dense" suffixes. Each of the 8 cores takes 1/8 of every chunk's
dense suffix plus 8 of the 64 bands via core_id-dependent dynamic APs.

Transport: ONE bf16 [268,1040] tensor per core (column shard of the packed
[268,8320] G = [4·x̂ᵀ ; F1(6) ; F2(6)]), AllGathered on-device. Label masks
come from a rank-6 matmul computing P = 64(Δa)²+64(Δb)² from base-16 digits
of the run index mod 256 (bf16-exact; Δrun ≤ 255 within any band window, so
P==0 ⇔ same label). Per-core exp-sums are reduced on-device to a [1,2]
scalar pair, AllReduced, and only core 0's shard is fetched. The host adds
exact closed-form corrections for the suppressed (masked) entries and takes
the final log-sum-exps.
"""

import sys
import numpy as np

for _p in ("/opt/trn_rl_repo",):
    if _p not in sys.path:
        sys.path.append(_p)

B = 8192
D = 256
NCORE = 8
CH = 128
NCH = B // CH          # 64 row chunks
BAND = 256
BP = B + 128           # X padded to 8320 cols (chunk 63's band overruns)
SH = BP // NCORE       # 1040 cols per core shard
FP8 = True             # X rows in float8_e4m3; F rows ride along as raw bytes
GR = (D + 24) if FP8 else (D + 12)  # packed G rows per core
KT = 2                 # K tiles of 128 (D = 256)
NEG_SHIFT = 32.0       # exp(v - 32), v = z^2 = qn + 16
POS_SHIFT = 416.0      # exp(v - 416), v = m^2 = qp + 16
Z_HI = 11.0            # safety clamp: t>11 impossible for |sim|<0.69
M_LO = -22.0           # safety clamp on pos side

_RT = None
_SALT = 1.0  # bump to force a NEFF recompile (cache-bust)


def _dense_len(m):
    return max(0, 992 - 16 * m)


def _build():
    import concourse.bass as bass
    import concourse.bacc as bacc
    import concourse.tile as tile
    import concourse.mybir as mybir

    dt = mybir.dt
    Alu = mybir.AluOpType
    Act = mybir.ActivationFunctionType

    nc = bacc.Bacc("TRN2", target_bir_lowering=False, debug=False,
                   num_devices=NCORE)

    xdt = dt.float8e4 if FP8 else dt.bfloat16
    g_d = nc.dram_tensor("g", [GR, SH], xdt, kind="ExternalInput")
    u_d = nc.dram_tensor("u", [CH, BAND], dt.float32, kind="ExternalInput")
    reps_d = nc.dram_tensor("reps", [1, 1], dt.int32, kind="ExternalInput")
    out_d = nc.dram_tensor("out", [1, 2], dt.float32, kind="ExternalOutput")

    with tile.TileContext(nc) as tc:
        with (
            tc.tile_pool(name="dram", bufs=1, space="DRAM") as dram,
            tc.tile_pool(name="xp", bufs=1) as xp,
            tc.tile_pool(name="cst", bufs=1) as cst,
            tc.tile_pool(name="ps", bufs=2, space="PSUM") as psd,
            tc.tile_pool(name="psb", bufs=1, space="PSUM") as psb,
            tc.tile_pool(name="psr", bufs=1, space="PSUM") as psr,
            tc.tile_pool(name="zp", bufs=3) as zp,
            tc.tile_pool(name="vp", bufs=3) as vp,
            tc.tile_pool(name="ep", bufs=2) as ep,
            tc.tile_pool(name="bp", bufs=2) as bpool,
            tc.tile_pool(name="acc", bufs=1) as accp,
        ):
            reps_t = cst.tile([1, 1], dt.int32, tag="reps")
            nc.sync.dma_start(reps_t[:], reps_d[:])
            reps_regs = nc.alloc_registers("reps_r")
            nc.regs_load(reps_regs, reps_t[0:1, 0:1])
            reps_v = nc.snap(reps_regs, donate=True)

            ut = cst.tile([CH, BAND], dt.float32, tag="u")
            nc.sync.dma_start(ut[:], u_d[:])

            bneg = cst.tile([CH, 1], dt.float32, tag="bneg")
            bpos = cst.tile([CH, 1], dt.float32, tag="bpos")
            b64 = cst.tile([CH, 1], dt.float32, tag="b64")
            ones = cst.tile([CH, 1], dt.float32, tag="ones")
            salt = cst.tile([1, 1], dt.float32, tag="salt")
            nc.vector.memset(salt[:], _SALT)
            nc.vector.memset(bneg[:], -NEG_SHIFT)
            nc.vector.memset(bpos[:], -POS_SHIFT)
            nc.vector.memset(b64[:], 64.0)
            nc.vector.memset(ones[:], 1.0)

            # acc cols: 0..63 dense sn, 64..71 band sn, 72..79 band sp
            acc = accp.tile([CH, 80], dt.float32, tag="acc")
            nc.vector.memset(acc[:], 0.0)
            s2 = accp.tile([1, 2], dt.float32, tag="s2")

            # gather the per-core G shards into the full packed G
            g_bounce = dram.tile([GR, SH], xdt, tag="gb")
            gg = dram.tile([NCORE * GR, SH], xdt, tag="gg")
            nc.gpsimd.dma_start(g_bounce[:], g_d[:])
            nc.gpsimd.collective_compute(
                "AllGather", mybir.AluOpType.bypass,
                replica_groups=[list(range(NCORE))],
                ins=[g_bounce.opt()], outs=[gg.opt()],
            )

            loop = tc.For_i(0, reps_v, 1)
            loop.__enter__()
            xt = [xp.tile([CH, BP], xdt, name=f"xt{k}", tag=f"x{k}")
                  for k in range(KT)]
            f1t = xp.tile([6, BP], dt.bfloat16, tag="f1")
            f2t = xp.tile([6, BP], dt.bfloat16, tag="f2")
            if FP8:
                # F rows live as 24 raw byte-rows per block: reinterpret
                # pairs of fp8 rows as one bf16 row of 1040 elements
                ggr = gg[:].flatten().rearrange("(a b) -> a b",
                                                a=NCORE * GR // 2)
                h = GR // 2  # 140 rows per block in the half-view
            for c in range(NCORE):
                for k in range(KT):
                    nc.sync.dma_start(xt[k][:, c * SH:(c + 1) * SH],
                                      gg[GR * c + CH * k:GR * c + CH * (k + 1), :])
                if FP8:
                    nc.sync.dma_start(
                        f1t[:, c * SH:(c + 1) * SH],
                        ggr[h * c + CH:h * c + CH + 6, :].bitcast(dt.bfloat16))
                    nc.sync.dma_start(
                        f2t[:, c * SH:(c + 1) * SH],
                        ggr[h * c + CH + 6:h * c + CH + 12, :].bitcast(dt.bfloat16))
                else:
                    nc.sync.dma_start(f1t[:, c * SH:(c + 1) * SH],
                                      gg[GR * c + D:GR * c + D + 6, :])
                    nc.sync.dma_start(f2t[:, c * SH:(c + 1) * SH],
                                      gg[GR * c + D + 6:GR * c + D + 12, :])

            pid = nc.tensor.partition_id()
            pid_pool = nc.gpsimd.partition_id()

            def do_dense(m):
                L = _dense_len(m)
                if L == 0:
                    return
                base = CH * m
                doff = pid * L + (base + BAND)
                pd = psd.tile([CH, 1024], dt.float32, tag="pd")
                n0 = 0
                while n0 < L:
                    n = min(512, L - n0)
                    for k in range(KT):
                        nc.tensor.matmul(
                            pd[:, n0:n0 + n],
                            xt[k][:, base:base + CH],
                            xt[k][:, bass.ds(doff + n0, n)],
                            start=(k == 0), stop=(k == KT - 1),
                        )
                    n0 += n
                zt = zp.tile([CH, 1024], dt.float32, tag="z")
                nc.vector.tensor_scalar(zt[:, :L], pd[:, :L], -4.0, Z_HI,
                                        Alu.max, Alu.min)
                vt = vp.tile([CH, 1024], dt.float32, tag="v")
                nc.scalar.square(vt[:, :L], zt[:, :L])
                et = ep.tile([CH, 1024], dt.float32, tag="e")
                nc.scalar.activation(et[:, :L], vt[:, :L], Act.Exp,
                                     bias=bneg[:], scale=1.0,
                                     accum_out=acc[:, m:m + 1])

            def do_band(k8):
                boff = pid * CH + 1024 * k8
                boff_p = pid_pool * CH + 1024 * k8
                # stationary operands cannot use register offsets: stage the
                # lhsT slices into fixed tiles first
                xl = [bpool.tile([CH, CH], xdt, name=f"xl{k8}_{k}",
                                 tag=f"xl{k}") for k in range(KT)]
                for k in range(KT):
                    nc.gpsimd.tensor_copy(xl[k][:], xt[k][:, bass.ds(boff_p, CH)])
                fl = bpool.tile([6, CH], dt.bfloat16, tag="fl")
                nc.gpsimd.tensor_copy(fl[:], f1t[:, bass.ds(boff_p, CH)])
                pt = psb.tile([CH, BAND], dt.float32, tag="bT")
                ptp = psb.tile([CH, BAND], dt.float32, tag="bTP")
                pp = psb.tile([CH, BAND], dt.float32, tag="bP")
                for k in range(KT):
                    nc.tensor.matmul(pt[:], xl[k][:],
                                     xt[k][:, bass.ds(boff, BAND)],
                                     start=(k == 0), stop=(k == KT - 1))
                for k in range(KT):
                    nc.tensor.matmul(ptp[:], xl[k][:],
                                     xt[k][:, bass.ds(boff, BAND)],
                                     start=(k == 0), stop=False)
                nc.tensor.matmul(ptp[:], fl[0:6, :],
                                 f2t[0:6, bass.ds(boff, BAND)],
                                 start=False, stop=True)
                nc.tensor.matmul(pp[:], fl[0:6, :],
                                 f2t[0:6, bass.ds(boff, BAND)],
                                 start=True, stop=True)

                # neg: z = max(min(T,11) - (relu(64-P) + U), -4)
                mp = bpool.tile([CH, BAND], dt.float32, tag="mp")
                nc.scalar.activation(mp[:], pp[:], Act.Relu, bias=b64[:], scale=-1.0)
                macc = bpool.tile([CH, BAND], dt.float32, tag="macc")
                nc.gpsimd.tensor_add(macc[:], mp[:], ut[:])
                bn = bpool.tile([CH, BAND], dt.float32, tag="bn")
                nc.vector.scalar_tensor_tensor(bn[:], pt[:], Z_HI, macc[:],
                                               Alu.min, Alu.subtract)
                zb = bpool.tile([CH, BAND], dt.float32, tag="zb")
                nc.gpsimd.tensor_scalar_max(zb[:], bn[:], -4.0)
                vb = bpool.tile([CH, BAND], dt.float32, tag="vb")
                nc.scalar.square(vb[:], zb[:])
                eb = bpool.tile([CH, BAND], dt.float32, tag="eb")
                nc.scalar.activation(eb[:], vb[:], Act.Exp,
                                     bias=bneg[:], scale=1.0,
                                     accum_out=acc[:, 64 + k8:65 + k8])

                # pos: m = clamp(T + P - 16 + U, -22, 0)
                w2 = bpool.tile([CH, BAND], dt.float32, tag="w2")
                nc.vector.scalar_tensor_tensor(w2[:], ptp[:], -16.0, ut[:],
                                               Alu.add, Alu.add)
                mb = bpool.tile([CH, BAND], dt.float32, tag="mb")
                nc.gpsimd.tensor_scalar(mb[:], w2[:], 0.0, M_LO, Alu.min, Alu.max)
                vpb = bpool.tile([CH, BAND], dt.float32, tag="vpb")
                nc.scalar.square(vpb[:], mb[:])
                epb = bpool.tile([CH, BAND], dt.float32, tag="epb")
                nc.scalar.activation(epb[:], vpb[:], Act.Exp,
                                     bias=bpos[:], scale=1.0,
                                     accum_out=acc[:, 72 + k8:73 + k8])

            for m in range(NCH - 1, -1, -1):
                do_dense(m)
                if m % 8 == 0:
                    do_band(m // 8)

            # cross-partition reduce: [128,80] -> [1,80] -> two scalars
            ps1 = psr.tile([1, 80], dt.float32, tag="red")
            nc.tensor.matmul(ps1[:], ones[:], acc[:], start=True, stop=True)
            red_n = accp.tile([1, 72], dt.float32, tag="rn")
            red_p = accp.tile([1, 8], dt.float32, tag="rp")
            nc.scalar.activation(red_n[:], ps1[0:1, 0:72], Act.Identity,
                                 accum_out=s2[0:1, 0:1])
            nc.scalar.activation(red_p[:], ps1[0:1, 72:80], Act.Identity,
                                 accum_out=s2[0:1, 1:2])
            loop.__exit__(None, None, None)

            rin = dram.tile([1, 2], dt.float32, tag="rin")
            rout = dram.tile([1, 2], dt.float32, tag="rout")
            nc.sync.dma_start(rin[:], s2[:])
            nc.gpsimd.collective_compute(
                "AllReduce", mybir.AluOpType.add,
                replica_groups=[list(range(NCORE))],
                ins=[rin.opt()], outs=[rout.opt()],
            )
            nc.sync.dma_start(out_d[:], rout[:])

    nc.compile()
    return nc


class _Runtime:
    pass


def _get_rt():
    global _RT
    if _RT is not None:
        return _RT
    import jax
    import concourse.mybir as mybir
    from jax.sharding import Mesh, PartitionSpec, NamedSharding
    from jax.experimental.shard_map import shard_map
    from concourse.bass2jax import (
        _bass_exec_p, partition_id_tensor, install_neuronx_cc_hook)

    nc = _build()
    install_neuronx_cc_hook()

    partition_name = (nc.partition_id_tensor.name
                      if nc.partition_id_tensor else None)
    in_names, out_names, out_avals = [], [], []
    for alloc in nc.m.functions[0].allocations:
        if not isinstance(alloc, mybir.MemoryLocationSet):
            continue
        name = alloc.memorylocations[0].name
        if alloc.kind == "ExternalInput":
            if name != partition_name:
                in_names.append(name)
        elif alloc.kind == "ExternalOutput":
            out_names.append(name)
            shape = tuple(alloc.tensor_shape)
            dtype = mybir.dt.np(alloc.dtype)
            out_avals.append(jax.core.ShapedArray(shape, dtype))
    n_params = len(in_names)
    all_names = in_names + out_names
    if partition_name is not None:
        all_names = all_names + [partition_name]

    def _body(*args):
        operands = list(args)
        if partition_name is not None:
            operands.append(partition_id_tensor())
        outs = _bass_exec_p.bind(
            *operands,
            out_avals=tuple(out_avals),
            in_names=tuple(all_names),
            out_names=tuple(out_names),
            lowering_input_output_aliases=(),
            sim_require_finite=True,
            sim_require_nnan=True,
            nc=nc,
        )
        return tuple(outs)

    devices = jax.devices()[:NCORE]
    mesh = Mesh(np.asarray(devices), ("core",))
    in_specs = (PartitionSpec("core"),) * (n_params + len(out_names))
    out_specs = (PartitionSpec("core"),) * len(out_names)
    sharded = jax.jit(
        shard_map(_body, mesh=mesh, in_specs=in_specs, out_specs=out_specs,
                  check_rep=False),
        keep_unused=True,
    )

    shard0 = NamedSharding(mesh, PartitionSpec("core"))

    # staged constants (never change across calls; not donated)
    U = np.zeros((CH, BAND), np.float32)
    for p in range(CH):
        U[p, :p + 1] = 64.0
    u_const = jax.device_put(np.tile(U, (NCORE, 1)), shard0)
    zeros_const = [
        jax.device_put(np.zeros((NCORE * a.shape[0],) + a.shape[1:], a.dtype),
                       shard0)
        for a in out_avals
    ]

    rt = _Runtime()
    rt.jax = jax
    rt.nc = nc
    rt.sharded = sharded
    rt.shard0 = shard0
    rt.in_names = in_names
    rt.u_const = u_const
    rt.zeros_const = zeros_const
    rt.reps_cache = {}
    _RT = rt
    return rt


def _reps_const(rt, r):
    arr = rt.reps_cache.get(r)
    if arr is None:
        arr = rt.jax.device_put(
            np.tile(np.array([[r]], np.int32), (NCORE, 1)), rt.shard0)
        rt.reps_cache[r] = arr
    return arr


_F8_LUT = None
_PREP_BUF = None


def _f8_lut():
    global _F8_LUT
    if _F8_LUT is None:
        import ml_dtypes
        with np.errstate(invalid="ignore", over="ignore"):
            _F8_LUT = (np.arange(65536, dtype=np.uint16)
                       .view(np.float16)
                       .astype(ml_dtypes.float8_e4m3).view(np.uint8))
    return _F8_LUT


def _host_prep(embeddings, labels):
    import ml_dtypes
    bf16 = ml_dtypes.bfloat16
    f8 = ml_dtypes.float8_e4m3

    global _PREP_BUF
    emb = np.asarray(embeddings, np.float32)
    lab = np.asarray(labels)
    order = np.argsort(lab, kind="stable")
    lab_s = lab[order]
    scale = 4.0 / np.maximum(
        np.sqrt(np.einsum("ij,ij->i", emb, emb)), 1e-12)
    if _PREP_BUF is None:
        _PREP_BUF = np.empty((B, D), np.float32)
    emb_s = np.take(emb, order, axis=0, out=_PREP_BUF)
    emb_s *= np.take(scale, order)[:, None]

    # run index of each sorted row; digits of (run mod 256) in base 16
    r = np.zeros(B, np.int64)
    np.cumsum(lab_s[1:] != lab_s[:-1], out=r[1:])
    v = (r % 256).astype(np.float32)
    a = np.floor_divide(v, 16.0)
    b = v - 16.0 * a

    # F1 rows: [a^2, a, b^2, b, 1, 1];  F2 rows: [64, -128a, 64, -128b, 64a^2, 64b^2]
    # pad columns: label digits 20 (P >= 64*25 vs any real row)
    F = np.empty((12, BP), bf16)
    pad_vals = (400.0, 20.0, 400.0, 20.0, 1.0, 1.0,
                64.0, -2560.0, 64.0, -2560.0, 25600.0, 25600.0)
    rows = (a * a, a, b * b, b, None, None,
            None, -128.0 * a, None, -128.0 * b, 64.0 * a * a, 64.0 * b * b)
    consts = (None, None, None, None, 1.0, 1.0,
              64.0, None, 64.0, None, None, None)
    for i in range(12):
        if rows[i] is not None:
            F[i, 0:B] = rows[i]
        else:
            F[i, 0:B] = consts[i]
        F[i, B:] = pad_vals[i]

    if FP8:
        x8u = _f8_lut()[emb_s.T.astype(np.float16).view(np.uint16)]  # [D,B] u8
        Gu = np.empty((NCORE, GR, SH), np.uint8)
        Gx = Gu[:, 0:D, :]
        for c in range(NCORE):
            lo, hi = SH * c, min(B, SH * (c + 1))
            Gx[c, :, :hi - lo] = x8u[:, lo:hi]
            if hi - lo < SH:
                Gx[c, :, hi - lo:] = 0
        Gu[:, D:, :] = (F.view(np.uint8)
                        .reshape(12, NCORE, 2, SH).transpose(1, 0, 2, 3)
                        .reshape(NCORE, 24, SH))
        Gstack = Gu.reshape(NCORE * GR, SH).view(f8)
    else:
        G = np.zeros((GR, BP), bf16)
        G[0:D, 0:B] = emb_s.T.astype(bf16)
        G[D:D + 12, :] = F
        Gstack = np.ascontiguousarray(
            G.reshape(GR, NCORE, SH).transpose(1, 0, 2)).reshape(NCORE * GR, SH)

    # closed-form corrections for suppressed entries (host, float64):
    #   each suppressed band entry contributes e^{16-32} = e^{-16};
    #   count per row = (p+1) [diag-block j<=i] + cnt_same_upper;
    #   chunk 63's off-diag pad cols contribute 128*128 * e^{-32}.
    _, counts = np.unique(lab_s, return_counts=True)
    n_pos_upper = int(np.sum(counts * (counts - 1) // 2))
    n_diag = NCH * (CH * (CH + 1) // 2)
    corr = ((n_diag + n_pos_upper) * np.exp(np.float64(-16.0))
            + CH * CH * np.exp(np.float64(-32.0)))
    return Gstack, corr


def kernel(embeddings, labels, _reps=1):
    rt = _get_rt()
    Gstack, corr = _host_prep(embeddings, labels)
    vals = {"g": Gstack, "u": rt.u_const, "reps": _reps_const(rt, _reps)}
    args = [vals[n] for n in rt.in_names] + rt.zeros_const
    sn = sp = np.float64(-1.0)
    for _attempt in range(3):
        outs = rt.sharded(*args)
        o0 = np.asarray(outs[0].addressable_shards[0].data)
        sn = np.float64(o0[0, 0]) - corr
        sp = np.float64(o0[0, 1])
        if np.isfinite(sn) and np.isfinite(sp) and sn > 0 and sp > 0:
            break
    loss_n = np.log(2.0 * sn) + (NEG_SHIFT - 16.0)
    loss_p = np.log(2.0 * sp) + (POS_SHIFT - 16.0)
    z = loss_p + loss_n
    loss = z + np.log1p(np.exp(-z))
    return np.float32(loss)

